# revision 15
# baseline (speedup 1.0000x reference)
"""Correlation-layer kernel for Trainium2 (Bass/Tile), 8 NeuronCores.

Reference computation (n=4 views, scene batch b=8):
  x: [b*n, c=128, h=32, w=32] fp32
  f = x / (||x||_channels + eps)              (per-pixel L2 norm over c)
  corr[b,i,k,p,q] = <f[b, jj[i,k], :, p], f[b, i, :, q]>   (jj = off-diag views)
  out: [b*n*(n-1), hw=1024, 32, 32] fp32

Sharding: data-parallel over scenes — core s computes scene s's full
12 x [1024, 1024] cost volumes. No cross-core communication.
"""

import numpy as np

import concourse.bass as bass
import concourse.mybir as mybir
import concourse.tile as tile
from concourse import bacc
from concourse.bass_utils import run_bass_kernel_spmd

N_VIEWS = 4
C = 128           # channels == SBUF partitions
H = W = 32
HW = H * W        # 1024
B = 8             # scenes == cores
N_PAIRS = N_VIEWS * (N_VIEWS - 1)  # 12
EPS = 1e-8
QC = 512          # matmul moving-operand chunk (fp32 max free dim)
TG = 4            # p-tiles staged per output DMA (4 * 512KB = 2MB)

# off-diagonal view table: jj[i] = views j != i, in ascending order
JJ = [[j for j in range(N_VIEWS) if j != i] for i in range(N_VIEWS)]

F32 = mybir.dt.float32
F32R = mybir.dt.float32r


def _body(ctx, tc, out_ap, x_ap):
    nc = tc.nc
    xin = x_ap.rearrange("(v c) q -> c v q", c=C)            # [128, 4, 1024]
    outr = out_ap.rearrange("pr (t p) q -> pr p t q", p=C)   # [12, 128, 8, 1024]

    consts = ctx.enter_context(tc.tile_pool(name="consts", bufs=1))
    data = ctx.enter_context(tc.tile_pool(name="data", bufs=1))
    sqp = ctx.enter_context(tc.tile_pool(name="sq", bufs=2))
    nrmp = ctx.enter_context(tc.tile_pool(name="nrm", bufs=2))
    stage_p = ctx.enter_context(tc.tile_pool(name="stage", bufs=6))
    ps_mm = ctx.enter_context(tc.tile_pool(name="psmm", bufs=3, space="PSUM"))
    ps_nrm = ctx.enter_context(tc.tile_pool(name="psnrm", bufs=2, space="PSUM"))

    ones_f = consts.tile([C, C], F32)    # fp32 scratch of ones
    nc.vector.memset(ones_f, 1.0)
    ones_k = consts.tile([C, 1], F32R)   # lhsT for channel-sum (K=128, M=1)
    nc.vector.tensor_copy(ones_k, ones_f[:, 0:1])
    ones_m = consts.tile([1, C], F32R)   # lhsT for partition-broadcast (K=1, M=128)
    nc.vector.tensor_copy(ones_m, ones_f[0:1, :])

    xt = data.tile([C, N_VIEWS, HW], F32)   # raw input, c on partitions
    ft = data.tile([C, N_VIEWS, HW], F32R)   # normalized features

    dma_ctr = [0]

    def next_dma_eng():
        dma_ctr[0] += 1
        return nc.sync if dma_ctr[0] % 2 == 0 else nc.gpsimd

    def emit_input(v):
        for qc in range(HW // QC):
            s = slice(qc * QC, (qc + 1) * QC)
            next_dma_eng().dma_start(out=xt[:, v, s], in_=xin[:, v, s])

    # per-pixel L2 normalization over channels, 512-wide chunks end-to-end:
    # ss[p] = sum_c x^2 (ones-matmul) -> broadcast ss across partitions
    # (ones outer product) -> sqrt / +eps / ~1/x -> f = x * scale
    def emit_norm(v):
        sq = sqp.tile([C, HW], F32R)
        nb = nrmp.tile([C, HW], F32, tag="nb")
        sc = nrmp.tile([C, HW], F32, tag="sc")
        ss_sb = nrmp.tile([1, HW], F32R, tag="ss_sb")
        for qc in range(HW // QC):
            s = slice(qc * QC, (qc + 1) * QC)
            nc.scalar.activation(sq[:, s], xt[:, v, s],
                                 mybir.ActivationFunctionType.Square)
            ssp = ps_nrm.tile([C, QC], F32, tag="nrm")
            nc.tensor.matmul(ssp[0:1, :], ones_k, sq[:, s])
            nc.scalar.copy(ss_sb[:, s], ssp[0:1, :])
            bcp = ps_nrm.tile([C, QC], F32, tag="nrm")
            nc.tensor.matmul(bcp, ones_m, ss_sb[:, s])
            nc.scalar.activation(nb[:, s], bcp,
                                 mybir.ActivationFunctionType.Sqrt)
            # eps (1e-8) vs norm ~ sqrt(128) contributes < 1e-9 relative;
            # far below f32r matmul rounding, so 1/(norm+eps) ~= 1/norm.
            nc.vector.reciprocal_approx_fast(sc[:, s], nb[:, s])
            nc.vector.tensor_mul(ft[:, v, s], xt[:, v, s], sc[:, s])

    # cost volume for one ordered pair: corr[pr][p, q] = f_j[:, p] . f_i[:, q]
    def emit_pair(i, j):
        pr = i * (N_VIEWS - 1) + JJ[i].index(j)
        for tg in range(HW // C // TG):
            st = stage_p.tile([C, TG, HW], F32)
            for u in range(TG):
                p0 = (tg * TG + u) * C
                ps = ps_mm.tile([C, HW], F32)
                for qc in range(HW // QC):
                    s = slice(qc * QC, (qc + 1) * QC)
                    nc.tensor.matmul(ps[:, s], ft[:, j, p0:p0 + C],
                                     ft[:, i, s])
                # evacuate PSUM: split halves across DVE and ACT
                nc.vector.tensor_copy(st[:, u, 0:QC], ps[:, 0:QC])
                nc.scalar.copy(st[:, u, QC:HW], ps[:, QC:HW])
            next_dma_eng().dma_start(
                out=outr[pr, :, tg * TG:(tg + 1) * TG, :], in_=st)

    # interleave: pairs are emitted as soon as both their views are normalized,
    # so the output-DMA stream starts early and never starves.
    emit_input(0)
    emit_input(1)
    emit_norm(0)
    emit_norm(1)
    emit_pair(0, 1)
    emit_input(2)
    emit_norm(2)
    emit_pair(1, 0)
    emit_input(3)
    emit_norm(3)
    for i, j in [(0, 2), (2, 0), (1, 2), (2, 1),
                 (0, 3), (3, 0), (1, 3), (3, 1), (2, 3), (3, 2)]:
        emit_pair(i, j)


_NC_CACHE = {}


def _build():
    if "nc" in _NC_CACHE:
        return _NC_CACHE["nc"]
    nc = bacc.Bacc("TRN2", target_bir_lowering=False, debug=False,
                   num_devices=B)
    x = nc.dram_tensor("x", [N_VIEWS * C, HW], F32, kind="ExternalInput").ap()
    out = nc.dram_tensor("out", [N_PAIRS, HW, HW], F32,
                         kind="ExternalOutput").ap()
    from contextlib import ExitStack
    with tile.TileContext(nc) as tc, ExitStack() as ctx:
        _body(ctx, tc, out, x)
    nc.compile()
    _NC_CACHE["nc"] = nc
    return nc


def kernel(x):
    x = np.ascontiguousarray(np.asarray(x, dtype=np.float32))  # [32,128,32,32]
    xr = x.reshape(B, N_VIEWS * C, HW)
    nc = _build()
    in_maps = [{"x": np.ascontiguousarray(xr[s])} for s in range(B)]
    res = run_bass_kernel_spmd(nc, in_maps, core_ids=list(range(B))).results
    out = np.stack([res[s]["out"] for s in range(B)])  # [8, 12, 1024, 1024]
    return out.reshape(B * N_PAIRS, HW, H, W)


# revision 17
# speedup vs baseline: 1.0329x; 1.0329x over previous
"""Correlation-layer kernel for Trainium2 (Bass/Tile), 8 NeuronCores.

Reference computation (n=4 views, scene batch b=8):
  x: [b*n, c=128, h=32, w=32] fp32
  f = x / (||x||_channels + eps)              (per-pixel L2 norm over c)
  corr[b,i,k,p,q] = <f[b, jj[i,k], :, p], f[b, i, :, q]>   (jj = off-diag views)
  out: [b*n*(n-1), hw=1024, 32, 32] fp32

Sharding: data-parallel over scenes — core s computes scene s's full
12 x [1024, 1024] cost volumes. No cross-core communication.
"""

import numpy as np

import concourse.bass as bass
import concourse.mybir as mybir
import concourse.tile as tile
from concourse import bacc
from concourse.bass_utils import run_bass_kernel_spmd

N_VIEWS = 4
C = 128           # channels == SBUF partitions
H = W = 32
HW = H * W        # 1024
B = 8             # scenes == cores
N_PAIRS = N_VIEWS * (N_VIEWS - 1)  # 12
EPS = 1e-8
QC = 512          # matmul moving-operand chunk (fp32 max free dim)
TG = 4            # p-tiles staged per output DMA (4 * 512KB = 2MB)

# off-diagonal view table: jj[i] = views j != i, in ascending order
JJ = [[j for j in range(N_VIEWS) if j != i] for i in range(N_VIEWS)]

F32 = mybir.dt.float32
F32R = mybir.dt.float32r


def _body(ctx, tc, out_ap, x_ap):
    nc = tc.nc
    xin = x_ap.rearrange("(v c) q -> c v q", c=C)            # [128, 4, 1024]
    outr = out_ap.rearrange("pr (t p) q -> pr p t q", p=C)   # [12, 128, 8, 1024]

    consts = ctx.enter_context(tc.tile_pool(name="consts", bufs=1))
    data = ctx.enter_context(tc.tile_pool(name="data", bufs=1))
    sqp = ctx.enter_context(tc.tile_pool(name="sq", bufs=2))
    nrmp = ctx.enter_context(tc.tile_pool(name="nrm", bufs=2))
    stage_p = ctx.enter_context(tc.tile_pool(name="stage", bufs=6))
    ps_mm = ctx.enter_context(tc.tile_pool(name="psmm", bufs=3, space="PSUM"))
    ps_nrm = ctx.enter_context(tc.tile_pool(name="psnrm", bufs=2, space="PSUM"))

    ones_f = consts.tile([C, C], F32)    # fp32 scratch of ones
    nc.vector.memset(ones_f, 1.0)
    # lhsT of all-ones [128,128]: one matmul both reduces over channels and
    # broadcasts the sum to all 128 output partitions
    ones_kk = consts.tile([C, C], F32R)
    nc.vector.tensor_copy(ones_kk, ones_f)

    xt = data.tile([C, N_VIEWS, HW], F32)   # raw input, c on partitions
    ft = data.tile([C, N_VIEWS, HW], F32R)   # normalized features

    dma_ctr = [0]

    def next_dma_eng():
        dma_ctr[0] += 1
        return nc.sync if dma_ctr[0] % 2 == 0 else nc.gpsimd

    def emit_input(v):
        for qc in range(HW // QC):
            s = slice(qc * QC, (qc + 1) * QC)
            next_dma_eng().dma_start(out=xt[:, v, s], in_=xin[:, v, s])

    # per-pixel L2 normalization over channels, 512-wide chunks end-to-end:
    # ss[p] = sum_c x^2 (ones-matmul) -> broadcast ss across partitions
    # (ones outer product) -> sqrt / +eps / ~1/x -> f = x * scale
    def emit_norm(v):
        sq = sqp.tile([C, HW], F32R)
        nb = nrmp.tile([C, HW], F32, tag="nb")
        sc = nrmp.tile([C, HW], F32, tag="sc")
        for qc in range(HW // QC):
            s = slice(qc * QC, (qc + 1) * QC)
            nc.vector.tensor_mul(sq[:, s], xt[:, v, s], xt[:, v, s])
            bcp = ps_nrm.tile([C, QC], F32, tag="nrm")
            nc.tensor.matmul(bcp, ones_kk, sq[:, s])
            nc.scalar.activation(nb[:, s], bcp,
                                 mybir.ActivationFunctionType.Sqrt)
            # eps (1e-8) vs norm ~ sqrt(128) contributes < 1e-9 relative;
            # far below f32r matmul rounding, so 1/(norm+eps) ~= 1/norm.
            nc.vector.reciprocal_approx_fast(sc[:, s], nb[:, s])
            nc.vector.tensor_mul(ft[:, v, s], xt[:, v, s], sc[:, s])

    # cost volume for one ordered pair: corr[pr][p, q] = f_j[:, p] . f_i[:, q]
    def emit_pair(i, j):
        pr = i * (N_VIEWS - 1) + JJ[i].index(j)
        for tg in range(HW // C // TG):
            st = stage_p.tile([C, TG, HW], F32)
            for u in range(TG):
                p0 = (tg * TG + u) * C
                ps = ps_mm.tile([C, HW], F32)
                for qc in range(HW // QC):
                    s = slice(qc * QC, (qc + 1) * QC)
                    nc.tensor.matmul(ps[:, s], ft[:, j, p0:p0 + C],
                                     ft[:, i, s])
                # evacuate PSUM: split halves across DVE and ACT
                nc.vector.tensor_copy(st[:, u, 0:QC], ps[:, 0:QC])
                nc.scalar.copy(st[:, u, QC:HW], ps[:, QC:HW])
            next_dma_eng().dma_start(
                out=outr[pr, :, tg * TG:(tg + 1) * TG, :], in_=st)

    # interleave: pairs are emitted as soon as both their views are normalized,
    # so the output-DMA stream starts early and never starves.
    emit_input(0)
    emit_input(1)
    emit_norm(0)
    emit_norm(1)
    emit_pair(0, 1)
    emit_input(2)
    emit_norm(2)
    emit_pair(1, 0)
    emit_input(3)
    emit_norm(3)
    for i, j in [(0, 2), (2, 0), (1, 2), (2, 1),
                 (0, 3), (3, 0), (1, 3), (3, 1), (2, 3), (3, 2)]:
        emit_pair(i, j)


_NC_CACHE = {}


def _build():
    if "nc" in _NC_CACHE:
        return _NC_CACHE["nc"]
    nc = bacc.Bacc("TRN2", target_bir_lowering=False, debug=False,
                   num_devices=B)
    x = nc.dram_tensor("x", [N_VIEWS * C, HW], F32, kind="ExternalInput").ap()
    out = nc.dram_tensor("out", [N_PAIRS, HW, HW], F32,
                         kind="ExternalOutput").ap()
    from contextlib import ExitStack
    with tile.TileContext(nc) as tc, ExitStack() as ctx:
        _body(ctx, tc, out, x)
    nc.compile()
    _NC_CACHE["nc"] = nc
    return nc


def kernel(x):
    x = np.ascontiguousarray(np.asarray(x, dtype=np.float32))  # [32,128,32,32]
    xr = x.reshape(B, N_VIEWS * C, HW)
    nc = _build()
    in_maps = [{"x": np.ascontiguousarray(xr[s])} for s in range(B)]
    res = run_bass_kernel_spmd(nc, in_maps, core_ids=list(range(B))).results
    out = np.stack([res[s]["out"] for s in range(B)])  # [8, 12, 1024, 1024]
    return out.reshape(B * N_PAIRS, HW, H, W)


# revision 18
# speedup vs baseline: 1.1920x; 1.1540x over previous
"""Correlation-layer kernel for Trainium2 (Bass/Tile), 8 NeuronCores.

Reference computation (n=4 views, scene batch b=8):
  x: [b*n, c=128, h=32, w=32] fp32
  f = x / (||x||_channels + eps)              (per-pixel L2 norm over c)
  corr[b,i,k,p,q] = <f[b, jj[i,k], :, p], f[b, i, :, q]>   (jj = off-diag views)
  out: [b*n*(n-1), hw=1024, 32, 32] fp32

Sharding: data-parallel over scenes — core s computes scene s's full
12 x [1024, 1024] cost volumes. No cross-core communication.
"""

import numpy as np

import concourse.bass as bass
import concourse.mybir as mybir
import concourse.tile as tile
from concourse import bacc
from concourse.bass_utils import run_bass_kernel_spmd

N_VIEWS = 4
C = 128           # channels == SBUF partitions
H = W = 32
HW = H * W        # 1024
B = 8             # scenes == cores
N_PAIRS = N_VIEWS * (N_VIEWS - 1)  # 12
EPS = 1e-8
QC = 512          # matmul moving-operand chunk (fp32 max free dim)
TG = 4            # p-tiles staged per output DMA (4 * 512KB = 2MB)

# off-diagonal view table: jj[i] = views j != i, in ascending order
JJ = [[j for j in range(N_VIEWS) if j != i] for i in range(N_VIEWS)]

F32 = mybir.dt.float32
F32R = mybir.dt.float32r


def _body(ctx, tc, out_ap, x_ap):
    nc = tc.nc
    xin = x_ap.rearrange("(v c) q -> c v q", c=C)            # [128, 4, 1024]
    outr = out_ap.rearrange("pr (t p) q -> pr p t q", p=C)   # [12, 128, 8, 1024]

    consts = ctx.enter_context(tc.tile_pool(name="consts", bufs=1))
    data = ctx.enter_context(tc.tile_pool(name="data", bufs=1))
    sqp = ctx.enter_context(tc.tile_pool(name="sq", bufs=2))
    nrmp = ctx.enter_context(tc.tile_pool(name="nrm", bufs=2))
    stage_p = ctx.enter_context(tc.tile_pool(name="stage", bufs=6))
    ps_mm = ctx.enter_context(tc.tile_pool(name="psmm", bufs=3, space="PSUM"))
    ps_nrm = ctx.enter_context(tc.tile_pool(name="psnrm", bufs=2, space="PSUM"))

    ones_f = consts.tile([C, C], F32)    # fp32 scratch of ones
    nc.vector.memset(ones_f, 1.0)
    # lhsT of all-ones [128,128]: one matmul both reduces over channels and
    # broadcasts the sum to all 128 output partitions
    ones_kk = consts.tile([C, C], F32R)
    nc.vector.tensor_copy(ones_kk, ones_f)

    xt = data.tile([C, N_VIEWS, HW], F32)   # raw input, c on partitions
    ft = data.tile([C, N_VIEWS, HW], F32R)   # normalized features

    def next_dma_eng():
        # HWDGE only: SWDGE (gpsimd) adds a Q7 drain at kernel end that can
        # linger 15-30us past the last transfer
        return nc.sync

    def emit_input(v):
        for qc in range(HW // QC):
            s = slice(qc * QC, (qc + 1) * QC)
            next_dma_eng().dma_start(out=xt[:, v, s], in_=xin[:, v, s])

    # per-pixel L2 normalization over channels, 512-wide chunks end-to-end:
    # ss[p] = sum_c x^2 (ones-matmul) -> broadcast ss across partitions
    # (ones outer product) -> sqrt / +eps / ~1/x -> f = x * scale
    def emit_norm(v):
        sq = sqp.tile([C, HW], F32R)
        nb = nrmp.tile([C, HW], F32, tag="nb")
        sc = nrmp.tile([C, HW], F32, tag="sc")
        for qc in range(HW // QC):
            s = slice(qc * QC, (qc + 1) * QC)
            nc.vector.tensor_mul(sq[:, s], xt[:, v, s], xt[:, v, s])
            bcp = ps_nrm.tile([C, QC], F32, tag="nrm")
            nc.tensor.matmul(bcp, ones_kk, sq[:, s])
            nc.scalar.activation(nb[:, s], bcp,
                                 mybir.ActivationFunctionType.Sqrt)
            # eps (1e-8) vs norm ~ sqrt(128) contributes < 1e-9 relative;
            # far below f32r matmul rounding, so 1/(norm+eps) ~= 1/norm.
            nc.vector.reciprocal_approx_fast(sc[:, s], nb[:, s])
            nc.vector.tensor_mul(ft[:, v, s], xt[:, v, s], sc[:, s])

    # cost volume for one ordered pair: corr[pr][p, q] = f_j[:, p] . f_i[:, q]
    def emit_pair(i, j):
        pr = i * (N_VIEWS - 1) + JJ[i].index(j)
        for tg in range(HW // C // TG):
            st = stage_p.tile([C, TG, HW], F32)
            for u in range(TG):
                p0 = (tg * TG + u) * C
                ps = ps_mm.tile([C, HW], F32)
                for qc in range(HW // QC):
                    s = slice(qc * QC, (qc + 1) * QC)
                    nc.tensor.matmul(ps[:, s], ft[:, j, p0:p0 + C],
                                     ft[:, i, s])
                # evacuate PSUM: split halves across DVE and ACT
                nc.vector.tensor_copy(st[:, u, 0:QC], ps[:, 0:QC])
                nc.scalar.copy(st[:, u, QC:HW], ps[:, QC:HW])
            next_dma_eng().dma_start(
                out=outr[pr, :, tg * TG:(tg + 1) * TG, :], in_=st)

    # interleave: pairs are emitted as soon as both their views are normalized,
    # so the output-DMA stream starts early and never starves.
    emit_input(0)
    emit_input(1)
    emit_norm(0)
    emit_norm(1)
    emit_pair(0, 1)
    emit_input(2)
    emit_norm(2)
    emit_pair(1, 0)
    emit_input(3)
    emit_norm(3)
    for i, j in [(0, 2), (2, 0), (1, 2), (2, 1),
                 (0, 3), (3, 0), (1, 3), (3, 1), (2, 3), (3, 2)]:
        emit_pair(i, j)


_NC_CACHE = {}


def _build():
    if "nc" in _NC_CACHE:
        return _NC_CACHE["nc"]
    nc = bacc.Bacc("TRN2", target_bir_lowering=False, debug=False,
                   num_devices=B)
    x = nc.dram_tensor("x", [N_VIEWS * C, HW], F32, kind="ExternalInput").ap()
    out = nc.dram_tensor("out", [N_PAIRS, HW, HW], F32,
                         kind="ExternalOutput").ap()
    from contextlib import ExitStack
    with tile.TileContext(nc) as tc, ExitStack() as ctx:
        _body(ctx, tc, out, x)
    nc.compile()
    _NC_CACHE["nc"] = nc
    return nc


def kernel(x):
    x = np.ascontiguousarray(np.asarray(x, dtype=np.float32))  # [32,128,32,32]
    xr = x.reshape(B, N_VIEWS * C, HW)
    nc = _build()
    in_maps = [{"x": np.ascontiguousarray(xr[s])} for s in range(B)]
    res = run_bass_kernel_spmd(nc, in_maps, core_ids=list(range(B))).results
    out = np.stack([res[s]["out"] for s in range(B)])  # [8, 12, 1024, 1024]
    return out.reshape(B * N_PAIRS, HW, H, W)


# revision 21
# speedup vs baseline: 1.2029x; 1.0092x over previous
"""Correlation-layer kernel for Trainium2 (Bass/Tile), 8 NeuronCores.

Reference computation (n=4 views, scene batch b=8):
  x: [b*n, c=128, h=32, w=32] fp32
  f = x / (||x||_channels + eps)              (per-pixel L2 norm over c)
  corr[b,i,k,p,q] = <f[b, jj[i,k], :, p], f[b, i, :, q]>   (jj = off-diag views)
  out: [b*n*(n-1), hw=1024, 32, 32] fp32

Sharding: data-parallel over scenes — core s computes scene s's full
12 x [1024, 1024] cost volumes. No cross-core communication.
"""

import numpy as np

import concourse.bass as bass
import concourse.mybir as mybir
import concourse.tile as tile
from concourse import bacc
from concourse.bass_utils import run_bass_kernel_spmd

N_VIEWS = 4
C = 128           # channels == SBUF partitions
H = W = 32
HW = H * W        # 1024
B = 8             # scenes == cores
N_PAIRS = N_VIEWS * (N_VIEWS - 1)  # 12
EPS = 1e-8
QC = 512          # matmul moving-operand chunk (fp32 max free dim)
TG = 4            # p-tiles staged per output DMA (4 * 512KB = 2MB)

# off-diagonal view table: jj[i] = views j != i, in ascending order
JJ = [[j for j in range(N_VIEWS) if j != i] for i in range(N_VIEWS)]

F32 = mybir.dt.float32
F32R = mybir.dt.float32r


def _body(ctx, tc, out_ap, x_ap):
    nc = tc.nc
    xin = x_ap.rearrange("(v c) q -> c v q", c=C)            # [128, 4, 1024]
    outr = out_ap.rearrange("pr (t p) q -> pr p t q", p=C)   # [12, 128, 8, 1024]

    consts = ctx.enter_context(tc.tile_pool(name="consts", bufs=1))
    data = ctx.enter_context(tc.tile_pool(name="data", bufs=1))
    sqp = ctx.enter_context(tc.tile_pool(name="sq", bufs=2))
    nrmp = ctx.enter_context(tc.tile_pool(name="nrm", bufs=2))
    stage_p = ctx.enter_context(tc.tile_pool(name="stage", bufs=6))
    ps_mm = ctx.enter_context(tc.tile_pool(name="psmm", bufs=3, space="PSUM"))
    ps_nrm = ctx.enter_context(tc.tile_pool(name="psnrm", bufs=2, space="PSUM"))

    ones_f = consts.tile([C, C], F32)    # fp32 scratch of ones
    nc.vector.memset(ones_f, 1.0)
    # lhsT of all-ones [128,128]: one matmul both reduces over channels and
    # broadcasts the sum to all 128 output partitions
    ones_kk = consts.tile([C, C], F32R)
    nc.vector.tensor_copy(ones_kk, ones_f)

    xt = data.tile([C, N_VIEWS, HW], F32)   # raw input, c on partitions
    ft = data.tile([C, N_VIEWS, HW], F32R)   # normalized features

    def next_dma_eng():
        # HWDGE only: SWDGE (gpsimd) adds a Q7 drain at kernel end that can
        # linger 15-30us past the last transfer
        return nc.sync

    def emit_input(v):
        for qc in range(HW // QC):
            s = slice(qc * QC, (qc + 1) * QC)
            next_dma_eng().dma_start(out=xt[:, v, s], in_=xin[:, v, s])

    # per-pixel L2 normalization over channels, 512-wide chunks end-to-end:
    # ss[p] = sum_c x^2 (ones-matmul) -> broadcast ss across partitions
    # (ones outer product) -> sqrt / +eps / ~1/x -> f = x * scale
    def emit_norm(v, sq_on_act=False):
        sq = sqp.tile([C, HW], F32R)
        nb = nrmp.tile([C, HW], F32, tag="nb")
        sc = nrmp.tile([C, HW], F32, tag="sc")
        for qc in range(HW // QC):
            s = slice(qc * QC, (qc + 1) * QC)
            if sq_on_act:
                # ACT is idle during the prologue; keep DVE's early queue
                # short so the first PSUM evacuations start sooner
                nc.scalar.activation(sq[:, s], xt[:, v, s],
                                     mybir.ActivationFunctionType.Square)
            else:
                nc.vector.tensor_mul(sq[:, s], xt[:, v, s], xt[:, v, s])
            bcp = ps_nrm.tile([C, QC], F32, tag="nrm")
            nc.tensor.matmul(bcp, ones_kk, sq[:, s])
            nc.scalar.activation(nb[:, s], bcp,
                                 mybir.ActivationFunctionType.Sqrt)
            # eps (1e-8) vs norm ~ sqrt(128) contributes < 1e-9 relative;
            # far below f32r matmul rounding, so 1/(norm+eps) ~= 1/norm.
            nc.vector.reciprocal_approx_fast(sc[:, s], nb[:, s])
            nc.vector.tensor_mul(ft[:, v, s], xt[:, v, s], sc[:, s])

    # cost volume for one ordered pair: corr[pr][p, q] = f_j[:, p] . f_i[:, q]
    def emit_pair(i, j, tgs=TG):
        pr = i * (N_VIEWS - 1) + JJ[i].index(j)
        for tg in range(HW // C // tgs):
            st = stage_p.tile([C, tgs, HW], F32, tag="st")
            for u in range(tgs):
                p0 = (tg * tgs + u) * C
                ps = ps_mm.tile([C, HW], F32)
                for qc in range(HW // QC):
                    s = slice(qc * QC, (qc + 1) * QC)
                    nc.tensor.matmul(ps[:, s], ft[:, j, p0:p0 + C],
                                     ft[:, i, s])
                # evacuate PSUM: split halves across DVE and ACT
                nc.vector.tensor_copy(st[:, u, 0:QC], ps[:, 0:QC])
                nc.scalar.copy(st[:, u, QC:HW], ps[:, QC:HW])
            next_dma_eng().dma_start(
                out=outr[pr, :, tg * tgs:(tg + 1) * tgs, :], in_=st)

    # interleave: pairs are emitted as soon as both their views are normalized,
    # so the output-DMA stream starts early and never starves.
    emit_input(0)
    emit_input(1)
    emit_norm(0, sq_on_act=True)
    emit_norm(1, sq_on_act=True)
    emit_pair(0, 1, tgs=2)
    emit_input(2)
    emit_norm(2)
    emit_pair(1, 0)
    emit_input(3)
    emit_norm(3)
    for i, j in [(0, 2), (2, 0), (1, 2), (2, 1),
                 (0, 3), (3, 0), (1, 3), (3, 1), (2, 3), (3, 2)]:
        emit_pair(i, j)


_NC_CACHE = {}


def _build():
    if "nc" in _NC_CACHE:
        return _NC_CACHE["nc"]
    nc = bacc.Bacc("TRN2", target_bir_lowering=False, debug=False,
                   num_devices=B)
    x = nc.dram_tensor("x", [N_VIEWS * C, HW], F32, kind="ExternalInput").ap()
    out = nc.dram_tensor("out", [N_PAIRS, HW, HW], F32,
                         kind="ExternalOutput").ap()
    from contextlib import ExitStack
    with tile.TileContext(nc) as tc, ExitStack() as ctx:
        _body(ctx, tc, out, x)
    nc.compile()
    _NC_CACHE["nc"] = nc
    return nc


def kernel(x):
    x = np.ascontiguousarray(np.asarray(x, dtype=np.float32))  # [32,128,32,32]
    xr = x.reshape(B, N_VIEWS * C, HW)
    nc = _build()
    in_maps = [{"x": np.ascontiguousarray(xr[s])} for s in range(B)]
    res = run_bass_kernel_spmd(nc, in_maps, core_ids=list(range(B))).results
    out = np.stack([res[s]["out"] for s in range(B)])  # [8, 12, 1024, 1024]
    return out.reshape(B * N_PAIRS, HW, H, W)


# revision 25
# speedup vs baseline: 1.2079x; 1.0042x over previous
"""Correlation-layer kernel for Trainium2 (Bass/Tile), 8 NeuronCores.

Reference computation (n=4 views, scene batch b=8):
  x: [b*n, c=128, h=32, w=32] fp32
  f = x / (||x||_channels + eps)              (per-pixel L2 norm over c)
  corr[b,i,k,p,q] = <f[b, jj[i,k], :, p], f[b, i, :, q]>   (jj = off-diag views)
  out: [b*n*(n-1), hw=1024, 32, 32] fp32

Sharding: data-parallel over scenes — core s computes scene s's full
12 x [1024, 1024] cost volumes. No cross-core communication.
"""

import numpy as np

import concourse.bass as bass
import concourse.mybir as mybir
import concourse.tile as tile
from concourse import bacc
from concourse.bass_utils import run_bass_kernel_spmd

N_VIEWS = 4
C = 128           # channels == SBUF partitions
H = W = 32
HW = H * W        # 1024
B = 8             # scenes == cores
N_PAIRS = N_VIEWS * (N_VIEWS - 1)  # 12
EPS = 1e-8
QC = 512          # matmul moving-operand chunk (fp32 max free dim)
TG = 4            # p-tiles staged per output DMA (4 * 512KB = 2MB)

# off-diagonal view table: jj[i] = views j != i, in ascending order
JJ = [[j for j in range(N_VIEWS) if j != i] for i in range(N_VIEWS)]

F32 = mybir.dt.float32
F32R = mybir.dt.float32r


def _body(ctx, tc, out_ap, x_ap):
    nc = tc.nc
    xin = x_ap.rearrange("(v c) q -> c v q", c=C)            # [128, 4, 1024]
    outr = out_ap.rearrange("pr (t p) q -> pr p t q", p=C)   # [12, 128, 8, 1024]

    consts = ctx.enter_context(tc.tile_pool(name="consts", bufs=1))
    data = ctx.enter_context(tc.tile_pool(name="data", bufs=1))
    sqp = ctx.enter_context(tc.tile_pool(name="sq", bufs=2))
    nrmp = ctx.enter_context(tc.tile_pool(name="nrm", bufs=2))
    stage_p = ctx.enter_context(tc.tile_pool(name="stage", bufs=6))
    ps_mm = ctx.enter_context(tc.tile_pool(name="psmm", bufs=3, space="PSUM"))
    ps_nrm = ctx.enter_context(tc.tile_pool(name="psnrm", bufs=2, space="PSUM"))

    ones_f = consts.tile([C, C], F32)    # fp32 scratch of ones
    nc.vector.memset(ones_f, 1.0)
    # lhsT of all-ones [128,128]: one matmul both reduces over channels and
    # broadcasts the sum to all 128 output partitions
    ones_kk = consts.tile([C, C], F32R)
    nc.vector.tensor_copy(ones_kk, ones_f)

    xt = data.tile([C, N_VIEWS, HW], F32)   # raw input, c on partitions
    ft = data.tile([C, N_VIEWS, HW], F32R)   # normalized features

    def next_dma_eng():
        # HWDGE only: SWDGE (gpsimd) adds a Q7 drain at kernel end that can
        # linger 15-30us past the last transfer
        return nc.sync

    def emit_input(v):
        for qc in range(HW // QC):
            s = slice(qc * QC, (qc + 1) * QC)
            next_dma_eng().dma_start(out=xt[:, v, s], in_=xin[:, v, s])

    # per-pixel L2 normalization over channels, 512-wide chunks end-to-end:
    # ss[p] = sum_c x^2 (ones-matmul) -> broadcast ss across partitions
    # (ones outer product) -> sqrt / +eps / ~1/x -> f = x * scale
    norm_tiles = {}

    def emit_norm_chunk(v, qc, sq_on_act=False):
        if v not in norm_tiles:
            norm_tiles[v] = (
                sqp.tile([C, HW], F32R, tag="sq", name=f"sq{v}"),
                nrmp.tile([C, HW], F32, tag="nb", name=f"nb{v}"),
                nrmp.tile([C, HW], F32, tag="sc", name=f"sc{v}"),
            )
        sq, nb, sc = norm_tiles[v]
        s = slice(qc * QC, (qc + 1) * QC)
        if sq_on_act:
            # ACT is idle during the prologue; keep DVE's early queue
            # short so the first PSUM evacuations start sooner
            nc.scalar.activation(sq[:, s], xt[:, v, s],
                                 mybir.ActivationFunctionType.Square)
        else:
            nc.vector.tensor_mul(sq[:, s], xt[:, v, s], xt[:, v, s])
        bcp = ps_nrm.tile([C, QC], F32, tag="nrm")
        nc.tensor.matmul(bcp, ones_kk, sq[:, s])
        nc.scalar.activation(nb[:, s], bcp,
                             mybir.ActivationFunctionType.Sqrt)
        # eps (1e-8) vs norm ~ sqrt(128) contributes < 1e-9 relative;
        # far below f32r matmul rounding, so 1/(norm+eps) ~= 1/norm.
        nc.vector.reciprocal_approx_fast(sc[:, s], nb[:, s])
        nc.vector.tensor_mul(ft[:, v, s], xt[:, v, s], sc[:, s])

    def emit_norm(v, sq_on_act=False):
        for qc in range(HW // QC):
            emit_norm_chunk(v, qc, sq_on_act)

    # cost volume for one ordered pair: corr[pr][p, q] = f_j[:, p] . f_i[:, q]
    def emit_pair(i, j, tgs=TG, stages=None):
        pr = i * (N_VIEWS - 1) + JJ[i].index(j)
        for tg in (range(HW // C // tgs) if stages is None else stages):
            st = stage_p.tile([C, tgs, HW], F32, tag="st")
            for u in range(tgs):
                p0 = (tg * tgs + u) * C
                ps = ps_mm.tile([C, HW], F32)
                for qc in range(HW // QC):
                    s = slice(qc * QC, (qc + 1) * QC)
                    nc.tensor.matmul(ps[:, s], ft[:, j, p0:p0 + C],
                                     ft[:, i, s])
                # evacuate PSUM: split halves across DVE and ACT
                nc.vector.tensor_copy(st[:, u, 0:QC], ps[:, 0:QC])
                nc.scalar.copy(st[:, u, QC:HW], ps[:, QC:HW])
            next_dma_eng().dma_start(
                out=outr[pr, :, tg * tgs:(tg + 1) * tgs, :], in_=st)

    # interleave: pairs are emitted as soon as both their views are normalized,
    # so the output-DMA stream starts early and never starves.
    emit_input(0)
    emit_input(1)
    emit_norm(0, sq_on_act=True)
    emit_norm_chunk(1, 0, sq_on_act=True)
    # stages 0-1 only touch p < 512 of view 1 (lhsT chunk 0), so they can
    # start before view 1's second chunk is normalized
    emit_pair(0, 1, tgs=2, stages=[0, 1])
    emit_norm_chunk(1, 1, sq_on_act=True)
    emit_pair(0, 1, tgs=2, stages=[2, 3])
    emit_input(2)
    emit_norm(2)
    emit_pair(1, 0)
    emit_input(3)
    emit_norm(3)
    for i, j in [(0, 2), (2, 0), (1, 2), (2, 1),
                 (0, 3), (3, 0), (1, 3), (3, 1), (2, 3), (3, 2)]:
        emit_pair(i, j)


_NC_CACHE = {}


def _build():
    if "nc" in _NC_CACHE:
        return _NC_CACHE["nc"]
    nc = bacc.Bacc("TRN2", target_bir_lowering=False, debug=False,
                   num_devices=B)
    x = nc.dram_tensor("x", [N_VIEWS * C, HW], F32, kind="ExternalInput").ap()
    out = nc.dram_tensor("out", [N_PAIRS, HW, HW], F32,
                         kind="ExternalOutput").ap()
    from contextlib import ExitStack
    with tile.TileContext(nc) as tc, ExitStack() as ctx:
        _body(ctx, tc, out, x)
    nc.compile()
    _NC_CACHE["nc"] = nc
    return nc


def kernel(x):
    x = np.ascontiguousarray(np.asarray(x, dtype=np.float32))  # [32,128,32,32]
    xr = x.reshape(B, N_VIEWS * C, HW)
    nc = _build()
    in_maps = [{"x": np.ascontiguousarray(xr[s])} for s in range(B)]
    res = run_bass_kernel_spmd(nc, in_maps, core_ids=list(range(B))).results
    out = np.stack([res[s]["out"] for s in range(B)])  # [8, 12, 1024, 1024]
    return out.reshape(B * N_PAIRS, HW, H, W)


# revision 26
# speedup vs baseline: 1.2122x; 1.0035x over previous
"""Correlation-layer kernel for Trainium2 (Bass/Tile), 8 NeuronCores.

Reference computation (n=4 views, scene batch b=8):
  x: [b*n, c=128, h=32, w=32] fp32
  f = x / (||x||_channels + eps)              (per-pixel L2 norm over c)
  corr[b,i,k,p,q] = <f[b, jj[i,k], :, p], f[b, i, :, q]>   (jj = off-diag views)
  out: [b*n*(n-1), hw=1024, 32, 32] fp32

Sharding: data-parallel over scenes — core s computes scene s's full
12 x [1024, 1024] cost volumes. No cross-core communication.
"""

import numpy as np

import concourse.bass as bass
import concourse.mybir as mybir
import concourse.tile as tile
from concourse import bacc
from concourse.bass_utils import run_bass_kernel_spmd

N_VIEWS = 4
C = 128           # channels == SBUF partitions
H = W = 32
HW = H * W        # 1024
B = 8             # scenes == cores
N_PAIRS = N_VIEWS * (N_VIEWS - 1)  # 12
EPS = 1e-8
QC = 512          # matmul moving-operand chunk (fp32 max free dim)
TG = 4            # p-tiles staged per output DMA (4 * 512KB = 2MB)

# off-diagonal view table: jj[i] = views j != i, in ascending order
JJ = [[j for j in range(N_VIEWS) if j != i] for i in range(N_VIEWS)]

F32 = mybir.dt.float32
F32R = mybir.dt.float32r


def _body(ctx, tc, out_ap, x_ap):
    nc = tc.nc
    xin = x_ap.rearrange("(v c) q -> c v q", c=C)            # [128, 4, 1024]
    outr = out_ap.rearrange("pr (t p) q -> pr p t q", p=C)   # [12, 128, 8, 1024]

    consts = ctx.enter_context(tc.tile_pool(name="consts", bufs=1))
    data = ctx.enter_context(tc.tile_pool(name="data", bufs=1))
    sqp = ctx.enter_context(tc.tile_pool(name="sq", bufs=2))
    nrmp = ctx.enter_context(tc.tile_pool(name="nrm", bufs=2))
    stage_p = ctx.enter_context(tc.tile_pool(name="stage", bufs=6))
    ps_mm = ctx.enter_context(tc.tile_pool(name="psmm", bufs=3, space="PSUM"))
    ps_nrm = ctx.enter_context(tc.tile_pool(name="psnrm", bufs=2, space="PSUM"))

    ones_f = consts.tile([C, C], F32)    # fp32 scratch of ones
    nc.vector.memset(ones_f, 1.0)
    # lhsT of all-ones [128,128]: one matmul both reduces over channels and
    # broadcasts the sum to all 128 output partitions
    ones_kk = consts.tile([C, C], F32R)
    nc.vector.tensor_copy(ones_kk, ones_f)

    xt = data.tile([C, N_VIEWS, HW], F32)   # raw input, c on partitions
    ft = data.tile([C, N_VIEWS, HW], F32R)   # normalized features

    def next_dma_eng():
        # HWDGE only: SWDGE (gpsimd) adds a Q7 drain at kernel end that can
        # linger 15-30us past the last transfer
        return nc.sync

    def emit_input(v):
        for qc in range(HW // QC):
            s = slice(qc * QC, (qc + 1) * QC)
            next_dma_eng().dma_start(out=xt[:, v, s], in_=xin[:, v, s])

    # per-pixel L2 normalization over channels, 512-wide chunks end-to-end:
    # ss[p] = sum_c x^2 (ones-matmul) -> broadcast ss across partitions
    # (ones outer product) -> sqrt / +eps / ~1/x -> f = x * scale
    norm_tiles = {}

    def emit_norm_chunk(v, qc, sq_on_act=False):
        if v not in norm_tiles:
            norm_tiles[v] = (
                sqp.tile([C, HW], F32R, tag="sq", name=f"sq{v}"),
                nrmp.tile([C, HW], F32, tag="nb", name=f"nb{v}"),
                nrmp.tile([C, HW], F32, tag="sc", name=f"sc{v}"),
            )
        sq, nb, sc = norm_tiles[v]
        s = slice(qc * QC, (qc + 1) * QC)
        if sq_on_act:
            # ACT is idle during the prologue; keep DVE's early queue
            # short so the first PSUM evacuations start sooner
            nc.scalar.activation(sq[:, s], xt[:, v, s],
                                 mybir.ActivationFunctionType.Square)
        else:
            nc.vector.tensor_mul(sq[:, s], xt[:, v, s], xt[:, v, s])
        bcp = ps_nrm.tile([C, QC], F32, tag="nrm")
        nc.tensor.matmul(bcp, ones_kk, sq[:, s])
        nc.scalar.activation(nb[:, s], bcp,
                             mybir.ActivationFunctionType.Sqrt)
        # eps (1e-8) vs norm ~ sqrt(128) contributes < 1e-9 relative;
        # far below f32r matmul rounding, so 1/(norm+eps) ~= 1/norm.
        nc.vector.reciprocal_approx_fast(sc[:, s], nb[:, s])
        nc.vector.tensor_mul(ft[:, v, s], xt[:, v, s], sc[:, s])

    def emit_norm(v, sq_on_act=False):
        for qc in range(HW // QC):
            emit_norm_chunk(v, qc, sq_on_act)

    # cost volume for one ordered pair: corr[pr][p, q] = f_j[:, p] . f_i[:, q]
    def emit_pair(i, j, tgs=TG, stages=None):
        pr = i * (N_VIEWS - 1) + JJ[i].index(j)
        for tg in (range(HW // C // tgs) if stages is None else stages):
            st = stage_p.tile([C, tgs, HW], F32, tag="st")
            for u in range(tgs):
                p0 = (tg * tgs + u) * C
                ps = ps_mm.tile([C, HW], F32)
                for qc in range(HW // QC):
                    s = slice(qc * QC, (qc + 1) * QC)
                    nc.tensor.matmul(ps[:, s], ft[:, j, p0:p0 + C],
                                     ft[:, i, s])
                # evacuate PSUM: split halves across DVE and ACT
                nc.vector.tensor_copy(st[:, u, 0:QC], ps[:, 0:QC])
                nc.scalar.copy(st[:, u, QC:HW], ps[:, QC:HW])
            next_dma_eng().dma_start(
                out=outr[pr, :, tg * tgs:(tg + 1) * tgs, :], in_=st)

    # interleave: pairs are emitted as soon as both their views are normalized,
    # so the output-DMA stream starts early and never starves.
    emit_input(0)
    emit_input(1)
    emit_norm(0)
    emit_norm_chunk(1, 0)
    # stages 0-1 only touch p < 512 of view 1 (lhsT chunk 0), so they can
    # start before view 1's second chunk is normalized
    emit_pair(0, 1, tgs=2, stages=[0, 1])
    emit_norm_chunk(1, 1)
    emit_pair(0, 1, tgs=2, stages=[2, 3])
    emit_input(2)
    emit_norm(2)
    emit_pair(1, 0)
    emit_input(3)
    emit_norm(3)
    for i, j in [(0, 2), (2, 0), (1, 2), (2, 1),
                 (0, 3), (3, 0), (1, 3), (3, 1), (2, 3), (3, 2)]:
        emit_pair(i, j)


_NC_CACHE = {}


def _build():
    if "nc" in _NC_CACHE:
        return _NC_CACHE["nc"]
    nc = bacc.Bacc("TRN2", target_bir_lowering=False, debug=False,
                   num_devices=B)
    x = nc.dram_tensor("x", [N_VIEWS * C, HW], F32, kind="ExternalInput").ap()
    out = nc.dram_tensor("out", [N_PAIRS, HW, HW], F32,
                         kind="ExternalOutput").ap()
    from contextlib import ExitStack
    with tile.TileContext(nc) as tc, ExitStack() as ctx:
        _body(ctx, tc, out, x)
    nc.compile()
    _NC_CACHE["nc"] = nc
    return nc


def kernel(x):
    x = np.ascontiguousarray(np.asarray(x, dtype=np.float32))  # [32,128,32,32]
    xr = x.reshape(B, N_VIEWS * C, HW)
    nc = _build()
    in_maps = [{"x": np.ascontiguousarray(xr[s])} for s in range(B)]
    res = run_bass_kernel_spmd(nc, in_maps, core_ids=list(range(B))).results
    out = np.stack([res[s]["out"] for s in range(B)])  # [8, 12, 1024, 1024]
    return out.reshape(B * N_PAIRS, HW, H, W)
